# revision 119
# baseline (speedup 1.0000x reference)
"""8-core TP attention kernel for Trainium2 (Bass/Tile).

Problem: B=2, T=S=2048, D=2048, N=16 q-heads, KH=8 kv-heads, H=128.
Sharding: TP over heads. Core c owns q-heads {2c, 2c+1}, kv-head c, and the
D-output slice [256c, 256(c+1)) of o_proj. Per-head attention outputs are
AllGathered (bf16) across cores; o_proj is sharded on its output dim so the
host just concatenates the 8 output slices.

v2 schedule: one global interleaved instruction stream keeps the PE
saturated end-to-end:
  consts -> P1(b0) t0..3 -> [P1(b0) t4..15 (x) chunks(0,*)]
  -> [P1(b1) (x) chunks(0,*)/(1,0..)] -> [chunks(1,*) (x) o_proj(0,*)]
  -> o_proj(1,*) tail.
Key tricks vs v1:
  * rms-norm 1/sqrt via exp(-0.5*ln(m)): Ln+Exp live in ONE scalar table
    set (natural_log_exp_and_others), so interleaving P1 epilogues with
    P2 exp() never thrashes the ~2.7us ACT table loads.
  * causal diagonal blocks are computed at [128 s, 512-128j t] granularity
    (col-offset subrange of the PSUM bank); within-tile causal masking is
    one DVE multiply by a precomputed 0/1 triangle (bf16) - no f32 bias
    adds, no bias tables, and nothing on the gpsimd queue (where AllGather
    triggers camp waiting for the CC stream).
  * scalar engine never switches ACT table sets: rms-norm 1/sqrt is a
    bitcast-float log2 approx fed through Exp (same set as the attention
    exp) plus one Newton step on DVE; Sqrt/Ln/Square would each pay
    ~2.7us/switch at every P1<->P2 interleave point (measured: 65 table
    loads, ~85us of scalar time).
  * softmax denominators: exp tiles accumulate ELEMENTWISE on DVE (bf16)
    and one ones-matmul per chunk-head reduces the accumulator - the PE
    no longer re-streams every exp tile for its column sums (-32us PE).
  * qkv PSUM is consumed by ONE scalar copy; AV matmuls trail logits by
    DEPTH blocks, DVE sum-accumulation by SUMLAG; AllGather input DMAs +
    triggers are issued a few steps late (tick clock + TRIG_GAP spacing)
    so no queue ever head-blocks on the collective stream; 1/sum
    broadcast via a rank-1 PE matmul instead of gpsimd
    partition_broadcast.
  * rope tables and intermediates in bf16 (2x DVE rate); rope split
    DVE/gpsimd halves; transposes lag their tile by two slots and go
    through one PSUM bank [128,3,128]; o_proj is head-quarter-pipelined
    behind its AllGather with both dh accumulators paired in one 2-bank
    PSUM tile; paired-head [128,2,512] exp/logits/norm tiles let single
    fat ACT ops cover both heads (-352-cycle fixed cost per op).
All matmuls bf16 (fp8 was evaluated: e4m3's ~3% per-element rounding lands
~3-5% on the output - over the 2e-2 bar - because dot-product relative
error does not average down with contraction length).
"""
import sys
import os

sys.path.insert(0, "/opt/trn_rl_repo")

# Provide antenv.axon_hooks (missing from the read-only antenv package on
# PYTHONPATH) so bass_utils can capture NTFF profiles under axon when
# trace=True. Degrades to a None hook (trace skipped) when the .so lacks the
# profile symbols.
if "antenv.axon_hooks" not in sys.modules:
    import types as _types

    _mod = _types.ModuleType("antenv.axon_hooks")

    def _default_ntff_hook():
        import contextlib
        import ctypes

        so_path = "/opt/axon/libaxon_pjrt.so"
        if not os.path.exists(so_path):
            return None
        lib = ctypes.CDLL(so_path)
        if not hasattr(lib, "axon_start_nrt_profile"):
            return None
        lib.axon_start_nrt_profile.argtypes = [
            ctypes.POINTER(ctypes.c_int64), ctypes.c_size_t]
        lib.axon_start_nrt_profile.restype = ctypes.c_int64
        lib.axon_stop_nrt_profile.argtypes = [ctypes.c_char_p]
        lib.axon_stop_nrt_profile.restype = ctypes.c_int64

        @contextlib.contextmanager
        def _hook(output_dir, device_ids):
            import jax
            jax.devices()
            if device_ids:
                ids = (ctypes.c_int64 * len(device_ids))(*device_ids)
                rc = lib.axon_start_nrt_profile(ids, len(device_ids))
            else:
                rc = lib.axon_start_nrt_profile(None, 0)
            if rc != 0:
                raise RuntimeError(f"axon_start_nrt_profile rc={rc}")
            try:
                yield
            finally:
                n = lib.axon_stop_nrt_profile(str(output_dir).encode())
                if n < 0:
                    raise RuntimeError(f"axon_stop_nrt_profile rc={n}")
                print(f"profile: {n} file(s) written to {output_dir}")

        return _hook

    _mod._HOOK = None

    def _set_hook(hook, _m=_mod):
        _m._HOOK = hook

    def _get_hook(_m=_mod):
        if _m._HOOK is None:
            _m._HOOK = _default_ntff_hook()
        return _m._HOOK

    _mod.set_axon_ntff_profile_hook = _set_hook
    _mod.get_axon_ntff_profile_hook = _get_hook
    sys.modules["antenv.axon_hooks"] = _mod
    try:
        import antenv as _antenv
        _antenv.axon_hooks = _mod
    except ImportError:
        pass

import math
import numpy as np

B, T, D = 2, 2048, 2048
N, KH, H = 16, 8, 128
S = 2048
EPS = 1e-6
ROPE_THETA = 1000000.0
K_MASK = -0.7 * float(np.finfo(np.float32).max)
NCORES = 8
GLOC = N // NCORES        # 2 local q heads
DLOC = D // NCORES        # 256 output cols per core
NTT = T // 128            # 16 t-tiles
NTC = T // 512            # 4 t-chunks
NDC = D // 128            # 16 d-chunks
NSB = S // 128            # 16 s-blocks
HSCALE = float(H) ** -0.5


def _num_left_pad(seg):
    return np.sum(np.cumsum(seg != 0, axis=-1) == 0, axis=-1).astype(np.int32)


def _positions_from_segment_ids(seg):
    t = seg.shape[1]
    pos = np.arange(t, dtype=np.int32)[None, :] - np.argmax(seg, axis=1)[:, None]
    return np.where(seg != 0, pos, 2 ** 30)


def _host_mask_and_rope(x, q_norm_w, k_norm_w, segment_ids, start_ind, cur_ind):
    """Reproduce the reference mask / positions / rope tables in numpy."""
    b, t = segment_ids.shape
    s = S
    start = np.where(start_ind < 0, _num_left_pad(segment_ids), start_ind).astype(np.int64)
    pos = _positions_from_segment_ids(segment_ids).astype(np.int64) + int(cur_ind)

    fraction = np.arange(0, H, 2, dtype=np.float32) / np.float32(H)
    inv_freq = (1.0 / (np.float32(ROPE_THETA) ** fraction)).astype(np.float32)
    sinusoid = (pos.astype(np.float32)[:, :, None] * inv_freq[None, None, :]).astype(np.float32)
    sin, cos = np.sin(sinusoid).astype(np.float32), np.cos(sinusoid).astype(np.float32)

    q_pos = int(cur_ind) + np.arange(t, dtype=np.int64)[None, :] - start[:, None]
    ts_ = np.arange(s, dtype=np.int64)
    kv_seg = (ts_[None, :] >= start[:, None]) & (ts_[None, :] < int(cur_ind) + t)
    k_pos = ts_[None, :] - start[:, None]
    causal = k_pos[:, None, :] <= q_pos[:, :, None]
    seg_mask = kv_seg[:, None, :].astype(segment_ids.dtype) == segment_ids[:, :, None]
    final_mask = causal & seg_mask  # [B, T, S]
    return final_mask, sin, cos


def _numpy_reference(x, q_w, k_w, v_w, o_w, q_norm_w, k_norm_w, k_cache, v_cache,
                     segment_ids, start_ind, cur_ind):
    """Exact-ish numpy fallback (used only for non-structural inputs)."""
    def rms_norm(v, w):
        rms = np.sqrt(np.mean(v.astype(np.float32) ** 2, axis=-1, keepdims=True) + EPS)
        return (w * v / rms).astype(v.dtype)

    mask, sin, cos = _host_mask_and_rope(x, q_norm_w, k_norm_w, segment_ids,
                                         start_ind, cur_ind)

    q = rms_norm(np.einsum('BTD,DNH->BTNH', x, q_w), q_norm_w)
    k = rms_norm(np.einsum('BSD,DKH->BSKH', x, k_w), k_norm_w)
    v = np.einsum('BSD,DKH->BSKH', x, v_w)

    def rope(z):
        h = z.shape[-1] // 2
        z1, z2 = z[..., :h], z[..., h:]
        s_, c_ = sin[:, :, None, :], cos[:, :, None, :]
        return np.concatenate([z1 * c_ - z2 * s_, z2 * c_ + z1 * s_], axis=-1).astype(z.dtype)

    q, k = rope(q), rope(k)
    kc = np.array(k_cache)
    vc = np.array(v_cache)
    ci = int(cur_ind)
    kc[:, ci:ci + T] = k
    vc[:, ci:ci + T] = v

    b, t = x.shape[0], x.shape[1]
    qg = q.reshape(b, t, KH, N // KH, H)
    logits = np.einsum('BTHGD,BSHD->BHGTS', qg, kc) * HSCALE
    logits = np.where(mask[:, None, None, :, :], logits, np.float32(K_MASK))
    m = logits.max(axis=-1, keepdims=True)
    e = np.exp(logits - m)
    attn = (e / e.sum(axis=-1, keepdims=True)).astype(np.float32)
    o = np.einsum('BHGTS,BSHD->BTHGD', attn, vc).reshape(b, t, N, H)
    return np.einsum('BTNH,NHD->BTD', o, o_w).astype(np.float32)


LAST_RES = None


def _build_and_run(xT_r, w_all, o_w_bf, rope_sets, rope_idx):
    import ml_dtypes
    import concourse.bass as bass
    import concourse.mybir as mybir
    import concourse.tile as tile
    from concourse import bacc
    from concourse.bass_utils import run_bass_kernel_spmd
    from concourse.masks import make_identity

    F32 = mybir.dt.float32
    BF16 = mybir.dt.bfloat16
    NRSETS = len(rope_sets)

    nc = bacc.Bacc("TRN2", target_bir_lowering=False, debug=False, num_devices=NCORES)
    eps_t = nc.alloc_sbuf_tensor("const-eps", [128, 1], F32)
    nc.gpsimd.memset(eps_t.ap(), float(EPS))
    nc.const_aps.aps[(F32, float(EPS))] = eps_t.ap()
    RSQ_BIAS = float(0.5 * math.log(2.0) * (127.0 + 0.0450))
    rsqb_t = nc.alloc_sbuf_tensor("const-rsqb", [128, 1], F32)
    nc.gpsimd.memset(rsqb_t.ap(), RSQ_BIAS)
    nc.const_aps.aps[(F32, RSQ_BIAS)] = rsqb_t.ap()

    # ---- external I/O ----
    xT_d = nc.dram_tensor("xT", [B, NTT, 128, NDC, 128], BF16, kind="ExternalInput").ap()
    w_d = nc.dram_tensor("w_all", [NDC, 128, 512], BF16, kind="ExternalInput").ap()
    ow_d = nc.dram_tensor("o_w", [N, 128, DLOC], BF16, kind="ExternalInput").ap()
    rope_d = nc.dram_tensor("rope", [NRSETS, 4, NTT, 128, 64], BF16,
                            kind="ExternalInput").ap()
    out_d = nc.dram_tensor("out", [2, 128, B * T], F32, kind="ExternalOutput").ap()

    # per-chunk collective buffers (a t-split of the last chunk was tried
    # here: no measurable gain, and it added risk - keep uniform chunks)
    ag_specs = [(b, tcc, 0, 512) for b in range(B) for tcc in range(NTC)]
    ag_in1 = {}
    ag_out1 = {}
    for (b, tcc, tlo, thi) in ag_specs:
        w_ = thi - tlo
        ag_in1[(b, tcc, tlo)] = nc.dram_tensor(
            f"agin_{b}_{tcc}_{tlo}", [GLOC * 128, w_], BF16)
        ag_out1[(b, tcc, tlo)] = nc.dram_tensor(
            f"agout_{b}_{tcc}_{tlo}", [N * 128, w_], BF16,
            addr_space="Shared")

    with tile.TileContext(nc) as tc:
        cpool = tc.alloc_tile_pool(name="const", bufs=1)
        spool = tc.alloc_tile_pool(name="store", bufs=1)

        # ---- persistent tiles ----
        w_sb = cpool.tile([128, NDC, 512], BF16, tag="w")
        w_eng = [nc.sync, nc.scalar, nc.gpsimd, nc.scalar]
        for kq in range(4):
            w_eng[kq].dma_start(out=w_sb[:, 4 * kq:4 * kq + 4, :], in_=bass.AP(
                w_d.tensor, kq * 4 * 128 * 512,
                [[512, 128], [128 * 512, 4], [1, 512]]))
        ow_sb = cpool.tile([128, N, DLOC], BF16, tag="ow")
        nc.gpsimd.dma_start(out=ow_sb[:], in_=bass.AP(
            ow_d.tensor, 0, [[DLOC, 128], [128 * DLOC, N], [1, DLOC]]))
        rope_sb = cpool.tile([128, NRSETS, 4, NTT, 64], BF16, tag="rope")
        for rs in range(NRSETS):
            nc.gpsimd.dma_start(out=rope_sb[:, rs], in_=bass.AP(
                rope_d.tensor, rs * (4 * NTT * 128 * 64),
                [[64, 128], [NTT * 128 * 64, 4], [128 * 64, NTT], [1, 64]]))
        ident = cpool.tile([128, 128], BF16, tag="ident")
        make_identity(nc, ident[:])
        ones_f = cpool.tile([128, 1], F32, tag="onesf")
        nc.vector.memset(ones_f[:], 1.0)
        ones = cpool.tile([128, 1], BF16, tag="ones")
        nc.vector.tensor_copy(ones[:], ones_f[:])
        ones_row = cpool.tile([1, 128], F32, tag="onesrow")
        nc.vector.memset(ones_row[:], 1.0)
        # multiplicative causal mask for diagonal tiles: keep t >= s
        tri01 = cpool.tile([128, 128], BF16, tag="tri01")
        nc.gpsimd.memset(tri01[:], 1.0)
        nc.gpsimd.affine_select(
            out=tri01[:], in_=tri01[:],
            compare_op=mybir.AluOpType.is_ge,
            fill=0.0, base=0,
            pattern=[[1, 128]], channel_multiplier=-1)

        qT = spool.tile([128, B, GLOC, NTT, 128], BF16, tag="qT")
        kT = spool.tile([128, B, NTT, 128], BF16, tag="kT")
        vS = spool.tile([128, B, NSB, 128], BF16, tag="vS")

        # tiny warm-up collectives keep the CC stream from paying its
        # ~40-80us cold-start ramp right before the first real AllGather
        dmy_in = [nc.dram_tensor(f"dmyin_{i}", [1, 1], BF16) for i in range(5)]
        dmy_out = [nc.dram_tensor(f"dmyout_{i}", [NCORES, 1], BF16,
                                  addr_space="Shared") for i in range(5)]
        warm_state = {"n": 0}

        def warmup_ag(src_ap):
            i = warm_state["n"]
            if i >= len(dmy_in):
                return
            nc.gpsimd.dma_start(out=dmy_in[i].ap(), in_=src_ap)
            nc.gpsimd.collective_compute(
                "AllGather", mybir.AluOpType.bypass,
                replica_groups=[list(range(NCORES))],
                ins=[dmy_in[i].ap()], outs=[dmy_out[i].ap()])
            warm_state["n"] = i + 1

        warmup_ag(ones[0:1, 0:1])

        # ---- phase pools (regions A-C); pool release is LIFO, so the two
        # P1 PSUM pools (released at region D) are allocated LAST ----
        # PSUM budget (8 banks): lgp 2x2-bank + opp 2 + pps 1 + tps 1 = 8
        # (softmax-sum scalars live in the lg pool; no separate smp pool)
        p1 = tc.alloc_tile_pool(name="p1sb", bufs=3)
        p2 = tc.alloc_tile_pool(name="p2sb", bufs=4)
        lgp_state = {"pool": tc.alloc_tile_pool(name="lgpsA", bufs=2, space="PSUM")}
        opp = tc.alloc_tile_pool(name="ops", bufs=2, space="PSUM")
        pps = tc.alloc_tile_pool(name="p1ps", bufs=1, space="PSUM")
        tps = tc.alloc_tile_pool(name="p1tp", bufs=1, space="PSUM")

        # ================= P1: projection tile =================
        # Split into proj (matmuls + epilogue math -> rot) and transposes.
        # Transposes for tile t are issued AFTER tile t+1's matmuls so the
        # long cross-engine epilogue latency chain never head-blocks the PE
        # queue. All qkv PSUM consumption is funneled through ONE scalar
        # copy (qf) so the single PSUM accumulator frees ~0.7us after the
        # matmuls and the next tile's projections start immediately.
        rot_pend = {}

        def p1_tile_proj(b, tt):
            ri = rope_idx[(b, 'q')]
            xt = p1.tile([128, NDC, 128], BF16, tag="xt", bufs=4)
            nsplit = 4 if (b == 0 and tt == 0) else 1
            for sp in range(nsplit):
                kw_ = NDC // nsplit
                in_ap = bass.AP(
                    xT_d.tensor,
                    (b * NTT + tt) * (128 * NDC * 128) + sp * kw_ * 128,
                    [[NDC * 128, 128], [128, kw_], [1, 128]],
                )
                nc.sync.dma_start(out=xt[:, sp * kw_:(sp + 1) * kw_, :], in_=in_ap)
            qkv = pps.tile([128, 512], F32, tag="qkv")
            for k in range(NDC):
                nc.tensor.matmul(qkv[:], xt[:, k, :], w_sb[:, k, :],
                                 start=(k == 0), stop=(k == NDC - 1))
            qf = p1.tile([128, 4, 128], F32, tag="qf", bufs=3)
            nc.scalar.copy(qf.rearrange("p a b -> p (a b)"), qkv[:])
            # v copy (scalar; from SBUF)
            nc.scalar.copy(vS[:, b, tt, :], qf[:, 3, :])
            # rms stats: sum-of-squares (DVE, SBUF inputs), then
            # rsqrt via bitcast log2 approx -> Exp (same ACT table set as
            # P2's exp; Ln/Sqrt/Square would thrash tables) + 1 Newton.
            accs = p1.tile([128, 4], F32, tag="accs", bufs=6)
            sq = p1.tile([128, 3, 128], F32, tag="sq", bufs=2)
            nc.vector.tensor_mul(sq[:], qf[:, 0:3, :], qf[:, 0:3, :])
            nc.vector.tensor_reduce(
                accs[:, 0:3], sq[:], axis=mybir.AxisListType.X,
                op=mybir.AluOpType.add)
            mh = p1.tile([128, 4], F32, tag="mh", bufs=6)
            nc.vector.tensor_scalar(
                out=mh[:, 0:3], in0=accs[:, 0:3],
                scalar1=1.0 / H, scalar2=float(EPS),
                op0=mybir.AluOpType.mult, op1=mybir.AluOpType.add)
            fi = p1.tile([128, 4], F32, tag="fi", bufs=6)
            nc.vector.tensor_copy(fi[:, 0:3],
                                  mh[:, 0:3].bitcast(mybir.dt.int32))
            y0t = p1.tile([128, 4], F32, tag="y0", bufs=6)
            nc.scalar.activation(
                y0t[:, 0:3], fi[:, 0:3],
                mybir.ActivationFunctionType.Exp,
                bias=float(0.5 * math.log(2.0) * (127.0 + 0.0450)),
                scale=float(-0.5 * math.log(2.0) / (1 << 23)))
            tn = p1.tile([128, 4], F32, tag="tn", bufs=6)
            nc.vector.tensor_mul(tn[:, 0:3], y0t[:, 0:3], y0t[:, 0:3])
            nc.vector.tensor_mul(tn[:, 0:3], tn[:, 0:3], mh[:, 0:3])
            nc.vector.tensor_scalar(
                out=tn[:, 0:3], in0=tn[:, 0:3], scalar1=-0.5, scalar2=1.5,
                op0=mybir.AluOpType.mult, op1=mybir.AluOpType.add)
            rcp = p1.tile([128, 4], F32, tag="rcp", bufs=6)
            nc.vector.tensor_mul(rcp[:, 0:3], tn[:, 0:3], y0t[:, 0:3])
            qs = p1.tile([128, 3, 128], BF16, tag="qs", bufs=3)
            for hd in range(3):
                nc.vector.tensor_scalar(
                    out=qs[:, hd, :],
                    in0=qf[:, hd, :],
                    scalar1=rcp[:, hd:hd + 1], scalar2=None,
                    op0=mybir.AluOpType.mult)
            # rope (fused: q0,q1,k share the table): first half on DVE,
            # second half on gpsimd (bf16 everywhere for 2x DVE rate)
            CA = rope_sb[:, ri, 0, tt, :].unsqueeze(1).broadcast_to([128, 3, 64])
            SA = rope_sb[:, ri, 1, tt, :].unsqueeze(1).broadcast_to([128, 3, 64])
            CB = rope_sb[:, ri, 2, tt, :].unsqueeze(1).broadcast_to([128, 3, 64])
            SB = rope_sb[:, ri, 3, tt, :].unsqueeze(1).broadcast_to([128, 3, 64])
            q1 = qs[:, 0:3, 0:64]
            q2 = qs[:, 0:3, 64:128]
            rot = p1.tile([128, 3, 128], BF16, tag="rot", bufs=3)
            t1 = p1.tile([128, 3, 64], BF16, tag="t1", bufs=3)
            t2 = p1.tile([128, 3, 64], BF16, tag="t2", bufs=3)
            nc.vector.tensor_mul(t1[:], q1, CA)
            nc.vector.tensor_mul(t2[:], q2, SB)
            nc.vector.tensor_sub(rot[:, 0:3, 0:64], t1[:], t2[:])
            t3 = p1.tile([128, 3, 64], BF16, tag="t3", bufs=3)
            t4 = p1.tile([128, 3, 64], BF16, tag="t4", bufs=3)
            nc.gpsimd.tensor_mul(t3[:], q2, CB)
            nc.gpsimd.tensor_mul(t4[:], q1, SA)
            nc.gpsimd.tensor_add(rot[:, 0:3, 64:128], t3[:], t4[:])
            rot_pend[(b, tt)] = rot

        def p1_tile_transpose(b, tt):
            rot = rot_pend.pop((b, tt))
            ptr = tps.tile([128, 3, 128], BF16, tag="ptr")
            nc.tensor.transpose(ptr[:, 2, :], rot[:, 2, :], ident[:])
            nc.vector.tensor_copy(kT[:, b, tt, :], ptr[:, 2, :])
            for g in range(GLOC):
                nc.tensor.transpose(ptr[:, g, :], rot[:, g, :], ident[:])
                nc.vector.tensor_copy(qT[:, b, g, tt, :], ptr[:, g, :])

        # ================= P2: chunk emitter =================
        # AV trails logits by DEPTH blocks; sums trail by SUMLAG so the
        # first sum/AV of a chunk never head-blocks the PE queue on the
        # previous chunk's normalization chain (PSUM WAR on smp/opp).
        DEPTH = 3
        SUMLAG = 5
        # deferred issuance with a build-time tick clock (~1.3us/tick).
        # AllGather triggers additionally respect TRIG_GAP ticks between
        # consecutive triggers so a trigger never camps on the gpsimd queue
        # waiting for the previous collective to drain the CC stream.
        TRIG_GAP = 16
        clock = {"t": 12, "ltrig": -100}
        deferred = []

        def tick_deferred(n=1, force=False):
            clock["t"] += n
            while deferred:
                mt, closure, is_trig = deferred[0]
                if not force:
                    if clock["t"] < mt:
                        break
                    if is_trig and clock["t"] < clock["ltrig"] + TRIG_GAP:
                        break
                deferred.pop(0)
                if is_trig:
                    clock["ltrig"] = max(clock["t"], clock["ltrig"] + TRIG_GAP
                                         if force else clock["t"])
                closure()

        def flush_deferred():
            tick_deferred(0, force=True)

        class ChunkEmitter:
            def __init__(self, b, tc_, tlo=0, thi=512, shared=None, idx=0):
                self.b = b
                self.tc = tc_
                self.tlo = tlo
                self.thi = thi
                self.shared = shared
                self.idx = idx
                # diag blocks first (full-width start), then full blocks
                self.blk = []
                for j in range(4):
                    sb = 4 * tc_ + j
                    if 128 * (j + 1) <= self.tlo:
                        self.blk.append(('full', sb, None))
                    elif 128 * j < self.thi:
                        self.blk.append(('diag', sb, j))
                self.blk += [('full', sb, None) for sb in range(4 * tc_)]
                self.n = len(self.blk)
                self.li = 0
                self.ai = 0
                self.si = 0
                self.norm_done = False
                self.exs = [None] * self.n
                if self.shared is None:
                    self.shared = {}
                self.o_ps = None
                self.sA = None
                self.sB = None
                self.qrhs = None

            # max p1 tile index (within batch) this chunk needs before any work
            def need_tiles(self):
                return 4 * self.tc + 4

            def done(self):
                return self.norm_done

            def _logits(self):
                i = self.li
                kind, sb, j = self.blk[i]
                b, tc_ = self.b, self.tc
                if self.qrhs is None:
                    self.qrhs = [
                        qT[:, b, g, 4 * tc_:4 * tc_ + 4, :].rearrange(
                            "p a b -> p (a b)")
                        for g in range(GLOC)]
                    # per-pass PSUM tiles: pool WAR ordering guarantees the
                    # previous pass's reads complete before these writes
                    self.o_ps = [
                        opp.tile([128, 512], F32, tag="o",
                                 name=f"o_{b}_{tc_}_{self.tlo}_{g}")
                        for g in range(GLOC)]
                    # (col-tiled tile_position sum pairing was tried twice -
                    # wrong results on HW even with separate banks)
                    self.sA = None
                    self.sB = None
                    # softmax-denominator partials accumulate elementwise on
                    # DVE (bf16, both heads in ONE [128,2,512] op); one
                    # ones-matmul per chunk-g at the end replaces a
                    # per-block PE stream of the whole exp tile
                    self.exacc = p2.tile([128, 2, 512], BF16, tag="exacc",
                                         bufs=2,
                                         name=f"exacc_{b}_{tc_}_{self.tlo}")
                tlo, thi = self.tlo, self.thi
                C = tlo if kind == 'full' else 128 * j
                ex2 = p2.tile([128, 2, 512], BF16, tag="ex", bufs=7)
                lg2 = lgp_state["pool"].tile([128, 2, 512], F32, tag="lg")
                for g in range(GLOC):
                    lg = lg2[:, g, :]
                    if kind == 'full':
                        nc.tensor.matmul(lg[:, tlo:thi], kT[:, b, sb, :],
                                         self.qrhs[g][:, tlo:thi],
                                         start=True, stop=True)
                    else:
                        nc.tensor.matmul(lg[:, C:C + 128], kT[:, b, sb, :],
                                         self.qrhs[g][:, C:C + 128],
                                         start=True, stop=True)
                        if thi > C + 128:
                            nc.tensor.matmul(lg[:, C + 128:thi], kT[:, b, sb, :],
                                             self.qrhs[g][:, C + 128:thi],
                                             start=True, stop=True)
                # ONE fat exp over both heads' banks: saves the 352-cycle
                # ACT fixed overhead per block (~23us scalar total)
                nc.scalar.activation(
                    ex2[:, :, C:thi], lg2[:, :, C:thi],
                    mybir.ActivationFunctionType.Exp,
                    bias=0.0, scale=HSCALE)
                if kind == 'diag':
                    # causal mask: ONE DVE multiply covers both heads
                    nc.vector.tensor_mul(
                        ex2[:, :, C:C + 128], ex2[:, :, C:C + 128],
                        tri01[:].unsqueeze(1).broadcast_to([128, 2, 128]))
                self.exs[i] = (ex2, C)
                self.li += 1

            def _av(self):
                i = self.ai
                kind, sb, j = self.blk[i]
                b, thi = self.b, self.thi
                first = (i == 0)
                last = (i == self.n - 1)
                ex2, C = self.exs[i]
                for g in range(GLOC):
                    nc.tensor.matmul(self.o_ps[g][:, C:thi], vS[:, b, sb, :],
                                     ex2[:, g, C:thi], start=first, stop=last,
                                     skip_group_check=True)
                self.ai += 1

            def _sum(self):
                i = self.si
                thi = self.thi
                ex2, C = self.exs[i]
                if i == 0:
                    nc.vector.tensor_copy(self.exacc[:, :, C:thi],
                                          ex2[:, :, C:thi])
                else:
                    nc.vector.tensor_add(self.exacc[:, :, C:thi],
                                         self.exacc[:, :, C:thi],
                                         ex2[:, :, C:thi])
                self.exs[i] = None
                self.si += 1

            def _norm_and_ag(self):
                # stage 1 inline: rec/pb/otn. The agin DMAs and the AG
                # trigger live on the gpsimd queue; issuing them a few pump
                # steps later means they never block that queue waiting for
                # otn (which would stall rope -> transposes -> PE).
                b, tc_, tlo, thi = self.b, self.tc, self.tlo, self.thi
                w_ = thi - tlo
                s2t = lgp_state["pool"].tile([2, 2, 512], F32, tag="lg")
                nc.tensor.matmul(s2t[0:1, 0, tlo:thi], ones[:],
                                 self.exacc[:, 0, tlo:thi],
                                 start=True, stop=True)
                nc.tensor.matmul(s2t[0:1, 1, tlo:thi], ones[:],
                                 self.exacc[:, 1, tlo:thi],
                                 start=True, stop=True)
                # paired norm chain (both heads per op where banks allow):
                # one reciprocal over both sum lanes, rank-1 bcast matmuls
                # into one 2-bank tile, ONE fat scalar copy, then per-head
                # otn multiplies (o_ps banks are separate).
                rec2 = p2.tile([1, 2, 512], F32, tag="rec", bufs=2)
                nc.vector.reciprocal_approx_fast(rec2[:, :, 0:w_],
                                                 s2t[0:1, :, tlo:thi])
                rcp2 = lgp_state["pool"].tile([128, 2, 512], F32, tag="lg")
                for g in range(GLOC):
                    nc.tensor.matmul(rcp2[:, g, 0:w_], ones_row[:],
                                     rec2[:, g, 0:w_], start=True, stop=True)
                rcb2 = p2.tile([128, 2, 512], F32, tag="rcb", bufs=2)
                nc.scalar.copy(rcb2[:, :, 0:w_], rcp2[:, :, 0:w_])
                otns = []
                for g in range(GLOC):
                    otn = p2.tile([128, 512], BF16, tag="otn", bufs=4)
                    nc.vector.tensor_mul(otn[:, 0:w_], self.o_ps[g][:, tlo:thi],
                                         rcb2[:, g, 0:w_])
                    otns.append(otn)

                def stage2():
                    for g in range(GLOC):
                        dst = ag_in1[(b, tc_, tlo)].ap()[g * 128:(g + 1) * 128, :]
                        nc.scalar.dma_start(out=dst, in_=otns[g][:, 0:w_])

                    def stage3():
                        nc.gpsimd.collective_compute(
                            "AllGather", mybir.AluOpType.bypass,
                            replica_groups=[list(range(NCORES))],
                            ins=[ag_in1[(b, tc_, tlo)].ap()],
                            outs=[ag_out1[(b, tc_, tlo)].ap()])

                    # last two triggers skip the spacing gate: nothing
                    # latency-critical remains on gpsimd by then
                    gate = self.idx < len(ag_specs) - 2
                    deferred.append([clock["t"] + 2, stage3, gate])

                deferred.append([clock["t"] + 2, stage2, False])
                self.norm_done = True

            def step(self):
                """Issue one pipeline step. Returns issued PE-cols estimate."""
                cost = 0
                if self.li < self.n:
                    self._logits()
                    cost += 1024
                    if self.li - self.ai > DEPTH:
                        self._av()
                        cost += 1024
                    if self.li - self.si > SUMLAG:
                        self._sum()
                        cost += 1024
                elif self.ai < self.n:
                    self._av()
                    cost += 1024
                    if self.si < self.n:
                        self._sum()
                        cost += 1024
                elif self.si < self.n:
                    self._sum()
                    cost += 1024
                elif not self.norm_done:
                    self._norm_and_ag()
                return cost

        # ================= P3: o_proj emitter =================
        p3_state = {"pool": None, "psum": None}

        class P3Emitter:
            def __init__(self, b, tc_, tlo=0, thi=512):
                self.b = b
                self.tc = tc_
                self.tlo = tlo
                self.w = thi - tlo
                self.unit = 0      # 0: DMAs, 1..4: matmul quarters
                self.oins = None
                self.outp = [None, None]

            def done(self):
                return self.unit > 4

            def step(self):
                # head-quarter pipelining: 4 oin DMAs (4 heads each); both
                # dh accumulators consume quarter q as soon as it lands, so
                # o_proj latency after the AllGather is ~one quarter-DMA.
                p3 = p3_state["pool"]
                p3p = p3_state["psum"]
                b, tc_, tlo, w_ = self.b, self.tc, self.tlo, self.w
                if self.unit == 0:
                    self.oins = []
                    for q in range(4):
                        oin = p3.tile([128, 4, 512], BF16, tag="oin", bufs=8)
                        src = bass.AP(
                            ag_out1[(b, tc_, tlo)].ap().tensor,
                            q * 4 * 128 * w_,
                            [[w_, 128], [128 * w_, 4], [1, w_]])
                        nc.sync.dma_start(out=oin[:, :, 0:w_], in_=src)
                        self.oins.append(oin)
                    self.unit = 1
                    return 0
                q = self.unit - 1      # quarter 0..3
                if q == 0:
                    # both dh accumulators in one 2-bank PSUM tile -> ONE
                    # fat osb copy at the end (same bank budget as 2x1)
                    self.outp2 = p3p.tile([128, 2, 512], F32, tag="op",
                                          name=f"op_{b}_{tc_}_{tlo}")
                for dh in range(2):
                    for jj in range(4):
                        nh = q * 4 + jj
                        nc.tensor.matmul(
                            self.outp2[:, dh, 0:w_],
                            ow_sb[:, nh, dh * 128:dh * 128 + 128],
                            self.oins[q][:, jj, 0:w_],
                            start=(nh == 0), stop=(nh == N - 1))
                if q == 3:
                    co = b * T + tc_ * 512 + tlo
                    osb2 = p3.tile([128, 2, 512], F32, tag="osb", bufs=2)
                    nc.scalar.copy(osb2[:, :, 0:w_], self.outp2[:, :, 0:w_])
                    for dh in range(2):
                        nc.scalar.dma_start(
                            out=out_d[dh, :, co: co + w_],
                            in_=osb2[:, dh, 0:w_])
                self.unit += 1
                return 4096

        # ================= the global schedule =================
        tiles = [(0, tt) for tt in range(NTT)] + [(1, tt) for tt in range(NTT)]
        chunks = [ChunkEmitter(b_, tc_, tlo_, thi_, idx=k)
                  for k, (b_, tc_, tlo_, thi_) in enumerate(ag_specs)]
        tiles_done = [0, 0]
        warm_marks = {}

        ci = 0

        def pump_chunks(budget):
            nonlocal ci
            spent = 0
            while ci < len(chunks) and spent < budget:
                ch = chunks[ci]
                if tiles_done[ch.b] < ch.need_tiles():
                    break
                c = ch.step()
                tick_deferred()
                spent += max(c, 256)
                if ch.done():
                    ci += 1
            return spent

        # regions A-C: P1 tiles interleaved with attention chunks;
        # transposes run two tiles behind their projection so the long
        # cross-engine epilogue chain never reaches the PE queue head.
        for ti, (b, tt) in enumerate(tiles):
            p1_tile_proj(b, tt)
            if ti > 1:
                pb_, pt_ = tiles[ti - 2]
                p1_tile_transpose(pb_, pt_)
                tiles_done[pb_] = pt_ + 1
            if ti in warm_marks:
                warmup_ag(kT[0:1, b, tt, 0:1])
            tick_deferred(3)
            if ti >= 4:
                pump_chunks(6144)
        p1_tile_transpose(*tiles[-2])
        p1_tile_transpose(*tiles[-1])
        tiles_done[1] = NTT

        # drain b0 chunks fully before pool swap (keeps bank budget exact)
        while ci < len(chunks) and chunks[ci].b == 0:
            pump_chunks(1 << 30)

        # region D: release P1 PSUM (2 banks) -> o_proj PSUM (2 banks)
        tps.release()
        pps.release()
        p3_state["psum"] = tc.alloc_tile_pool(name="p3ps", bufs=1, space="PSUM")
        p3_state["pool"] = tc.alloc_tile_pool(name="p3sb", bufs=4)

        p3s = [P3Emitter(b_, tc_, tlo_, thi_)
               for (b_, tc_, tlo_, thi_) in ag_specs]
        pi = 0

        # interleave remaining (b1) chunks with o_proj of landed AllGathers;
        # reserve the last four o_proj emitters (+4 guard) to fill the final
        # AllGather's latency window after the last chunk.
        while ci < len(chunks) or pi < len(p3s):
            if ci < len(chunks):
                pump_chunks(4096)
            else:
                flush_deferred()
            issued = 0
            while pi < len(p3s) and issued < 2:
                tgt = p3s[pi]
                tgt_idx = pi
                # +3 reserve: measured sweet spot (+2 starves the tail AG
                # window, +4/+5 starve the PE in region D)
                if tgt_idx + 3 < ci or ci >= len(chunks):
                    if tgt.step():
                        issued += 1
                    tick_deferred(2)
                    if tgt.done():
                        pi += 1
                else:
                    break

        flush_deferred()
        p3_state["pool"].release()
        p3_state["psum"].release()
        opp.release()
        lgp_state["pool"].release()
        p2.release()
        p1.release()
        spool.release()
        cpool.release()

    nc.compile()

    in_maps = []
    for c in range(NCORES):
        in_maps.append({
            "xT": xT_r,
            "w_all": w_all[c],
            "o_w": o_w_bf[c],
            "rope": rope_sets,
        })
    trace = bool(os.environ.get("BASS_TRACE"))
    res = run_bass_kernel_spmd(nc, in_maps, core_ids=list(range(NCORES)),
                               trace=trace)
    global LAST_RES
    LAST_RES = res
    return res


def kernel(x, q_w, k_w, v_w, o_w, q_norm_w, k_norm_w, k_cache, v_cache,
           segment_ids, start_ind, cur_ind, right_pads):
    x = np.asarray(x, dtype=np.float32)
    q_w = np.asarray(q_w, dtype=np.float32)
    k_w = np.asarray(k_w, dtype=np.float32)
    v_w = np.asarray(v_w, dtype=np.float32)
    o_w = np.asarray(o_w, dtype=np.float32)
    q_norm_w = np.asarray(q_norm_w, dtype=np.float32)
    k_norm_w = np.asarray(k_norm_w, dtype=np.float32)
    segment_ids = np.asarray(segment_ids)
    start_ind = np.asarray(start_ind)
    ci = int(np.asarray(cur_ind))

    mask, sin, cos = _host_mask_and_rope(x, q_norm_w, k_norm_w, segment_ids,
                                         start_ind, ci)

    # fast path requires a pure causal mask (the staged problem's structure)
    tril = np.tril(np.ones((T, S), dtype=bool))
    structural = (
        x.shape == (B, T, D) and ci == 0 and S == T
        and all(bool(np.array_equal(mask[b], tril)) for b in range(B))
        and float(np.sqrt(H) * np.abs(q_norm_w).max() * np.abs(k_norm_w).max()) < 80.0
    )
    if not structural:
        return _numpy_reference(x, q_w, k_w, v_w, o_w, q_norm_w, k_norm_w,
                                k_cache, v_cache, segment_ids, start_ind, ci)

    # ---- host-side data prep ----
    import ml_dtypes
    xT_blk = np.ascontiguousarray(
        x.reshape(B, NTT, 128, NDC, 128).transpose(0, 1, 4, 3, 2))
    xT_r = xT_blk.astype(ml_dtypes.bfloat16)

    w_all = []
    o_w_bf = []
    ow_flat = o_w.reshape(N * H, D)
    for c in range(NCORES):
        wc = np.concatenate([
            q_w[:, 2 * c:2 * c + 2, :].reshape(D, 2 * H),
            k_w[:, c, :],
            v_w[:, c, :],
        ], axis=1)                                             # [D, 512]
        w_all.append(wc.reshape(NDC, 128, 512).astype(ml_dtypes.bfloat16))
        oc = ow_flat[:, c * DLOC:(c + 1) * DLOC]               # [2048, 256]
        o_w_bf.append(np.ascontiguousarray(
            oc.reshape(N, 128, DLOC)).astype(ml_dtypes.bfloat16))

    # rope tables fused with norm weights: CA, SA, CB, SB each [T, 64] (bf16)
    # fast path requires q and k to share a table per batch (fused rope)
    rope_sets = []
    rope_key = {}
    rope_idx = {}
    for b in range(B):
        for kind, w in (('q', q_norm_w), ('k', k_norm_w)):
            CA = cos[b] * w[None, :64]
            SA = sin[b] * w[None, :64]
            CB = cos[b] * w[None, 64:]
            SB = sin[b] * w[None, 64:]
            arr = np.stack([CA, SA, CB, SB]).astype(np.float32)  # [4, T, 64]
            key = arr.tobytes()
            if key not in rope_key:
                rope_key[key] = len(rope_sets)
                rope_sets.append(arr.reshape(4, NTT, 128, 64))
            rope_idx[(b, kind)] = rope_key[key]
    if any(rope_idx[(b, 'q')] != rope_idx[(b, 'k')] for b in range(B)):
        return _numpy_reference(x, q_w, k_w, v_w, o_w, q_norm_w, k_norm_w,
                                k_cache, v_cache, segment_ids, start_ind, ci)
    rope_sets = np.stack(rope_sets).astype(ml_dtypes.bfloat16)

    res = _build_and_run(xT_r, w_all, o_w_bf, rope_sets, rope_idx)

    out = np.empty((B, T, D), dtype=np.float32)
    for c in range(NCORES):
        oc = np.asarray(res.results[c]["out"])                 # [2, 128, B*T]
        oc = oc.reshape(DLOC, B, T).transpose(1, 2, 0)         # [B, T, 256]
        out[:, :, c * DLOC:(c + 1) * DLOC] = oc
    return out


if __name__ == "__main__":
    import reference
    inputs = {k: np.asarray(v) for k, v in reference.setup_inputs().items()}
    got = kernel(**inputs)
    print("kernel output", got.shape, got.dtype)
